# revision 64
# baseline (speedup 1.0000x reference)
"""BKT (Bayesian Knowledge Tracing) forward pass on Trainium2, 8 NeuronCores.

The reference's chunked 32-trajectory scan is mathematically a 2-state HMM
forward pass. Per (sequence, t):
    alpha' = alpha @ (diag(o_t) @ Tr)      (row vector times matrix)
with o_s(t) = P(obs_t | L=s), Tr the 2x2 BKT transition matrix, and
    out_c(t) = log(alpha@pc) - log(alpha@1),  pc = [P(c|0), P(c|1)].

v2 design (per core, batch-parallel over 2048 sequences laid out as
128 partitions x 16 groups; every layout keeps the packed group dim g
innermost so fp16 tensor_tensor ops hit the DVE 2x perf mode, and every
access pattern merges to <=3 free dims for the TENSOR3D ISA):
  1. Host sends fp16 signed logits (corr-select pre-applied) and fp16
     plain logits; ACT sigmoids emit fp16 obs probs, split by step parity.
  2. W_t = o_t x (2*Tr) built fp16 on the Pool engine into even/odd-step
     tiles [s][s'][(c k)][g].
  3. Chunk products A_c over K=10 steps via a pairwise tree: one fused op
     computes all 5 pair products, 3 combine levels reduce them; fp16 2x.
  4. Short fp32 serial recursion over chunk matrices; one batched
     renormalize + fp16 recovery-seed cast per segment, all on DVE.
  5. Within-chunk recovery (fp16 2x, parallel over chunks) -> per-t alphas.
  6. Predictions q = num * recip(den) (num path on Pool); ACT computes
     Ln(q) and Ln(1-q) (scale=-1, bias=1) straight into the output tile.
All fp16 roundings were validated against the fp64 reference: max rel
err ~7e-3 vs the 2e-2 gate (fp16 errors are local; cross-chunk spine
error cancels in the num/den ratio).

Sharding: pure data-parallel over batch; parameter tables are gathered on
host (traffic-neutral input marshaling), all recurrences stay on-device.
"""

import numpy as np

import concourse.bass as bass
import concourse.bacc as bacc
import concourse.tile as tile
import concourse.mybir as mybir
from concourse._compat import with_exitstack

F32 = mybir.dt.float32
F16 = mybir.dt.float16
AF = mybir.ActivationFunctionType
OP = mybir.AluOpType

P = 128          # partitions
N_CORES = 8


def emit_bkt(nc, G, T, K, SEG):
    """Emit the BKT kernel for one core. Sequences = P*G.

    DRAM tensors:
      zll: (P, T, 2, G) f16   [(2c-1)*lg, -(2c-1)*ls] signed obs logits
      ull: (P, T, 2, G) f16   [lg, -ls] plain logits (for true-outcome probs)
      dyn: (P, 3, G) f32      [logit_pL, logit_pF, logit_pI0]
      out: (P, T, 2, G) f32   [log p(incorrect), log p(correct)]
    """
    assert T % SEG == 0 and SEG % K == 0 and K == 10
    NSEG = T // SEG
    CS = SEG // K          # chunks per segment
    CT = T // K            # total chunks
    KH = K // 2            # steps per parity
    CK = CS * KH           # parity-steps per segment

    zll_d = nc.dram_tensor("zll", [P, T, 2, G], F16, kind="ExternalInput")
    ull_d = nc.dram_tensor("ull", [P, T, 2, G], F16, kind="ExternalInput")
    dyn_d = nc.dram_tensor("dyn", [P, 3, G], F32, kind="ExternalInput")
    out_d = nc.dram_tensor("out", [P, T, 2, G], F32, kind="ExternalOutput")

    with tile.TileContext(nc) as tc:
        with (
            tc.tile_pool(name="singles", bufs=1) as singles,
            tc.tile_pool(name="io", bufs=2) as io,
            tc.tile_pool(name="work", bufs=1) as work,
            tc.tile_pool(name="actb", bufs=2) as actb,
            tc.tile_pool(name="wpool", bufs=3) as wpool,
            tc.tile_pool(name="inpool", bufs=3) as inpool,
            tc.tile_pool(name="qpool", bufs=2) as qpool,
        ):
            # ---- per-sequence constants ----
            dyn_t = singles.tile([P, 3, G], F32)
            nc.sync.dma_start(dyn_t[:], dyn_d[:])
            # Tr packed [s][s']: [[1-l, l], [f, 1-f]]; 1-sigmoid(x) = sigmoid(-x)
            Tpf = singles.tile([P, 2, 2, G], F32)   # [s][s'][g]
            nc.scalar.activation(Tpf[:, 0, 0, :], dyn_t[:, 0, :], AF.Sigmoid, scale=-1.0)
            nc.scalar.activation(Tpf[:, 0, 1, :], dyn_t[:, 0, :], AF.Sigmoid)
            nc.scalar.activation(Tpf[:, 1, 0, :], dyn_t[:, 1, :], AF.Sigmoid)
            nc.scalar.activation(Tpf[:, 1, 1, :], dyn_t[:, 1, :], AF.Sigmoid, scale=-1.0)
            # 2x-scaled fp16 transition table (keeps fp16 chunk products in range)
            Tp2 = singles.tile([P, 2, 2, G], F16)
            nc.vector.tensor_scalar_mul(Tp2[:], Tpf[:], 2.0)

            # chunk-start alphas, all chunks + final carry (fp32 spine)
            starts = singles.tile([P, CT + 1, 2, G], F32)
            nc.scalar.activation(starts[:, 0, 0, :], dyn_t[:, 2, :], AF.Sigmoid, scale=-1.0)
            nc.scalar.activation(starts[:, 0, 1, :], dyn_t[:, 2, :], AF.Sigmoid)

            obs = {}        # per-seg live tiles from phase A
            fin = {}        # per-seg out tiles awaiting store

            def pool_tt(out, in0, in1, op):
                nc.gpsimd.tensor_tensor(out, in0, in1, op)

            def phase_a(seg, nsplit=1):
                """Loads + observation sigmoids + W-build for segment seg."""
                s0 = seg * SEG
                zll = inpool.tile([P, SEG, 2, G], F16, tag="zll")
                ull = inpool.tile([P, SEG, 2, G], F16, tag="ull")
                # obs probs split by step parity: [s][(c k)][g]
                opE = actb.tile([P, 2, CK, G], F16, tag="opE")
                opO = actb.tile([P, 2, CK, G], F16, tag="opO")
                ptp = actb.tile([P, 2, K, CS, G], F16, tag="ptp")  # [s][k][c][g]
                # W split by step parity: [s][s'][(c k)][g], W = o_s * 2Tr[s][s'].
                # Triple-buffered: recovery(s) is the last We(s) reader, and
                # with 2 bufs the W-build of s+2 would stall behind it.
                We = wpool.tile([P, 2, 2, CK, G], F16, tag="We")
                Wo = wpool.tile([P, 2, 2, CK, G], F16, tag="Wo")
                zc = zll[:].rearrange("p (ck two) s g -> p ck two s g", two=2)
                bounds = [SEG * h // nsplit for h in range(nsplit + 1)]
                for h in range(nsplit):
                    a, b = bounds[h], bounds[h + 1]
                    nc.sync.dma_start(zll[:, a:b], zll_d[:, s0 + a : s0 + b, :, :])
                for h in range(nsplit):
                    a, b = bounds[h], bounds[h + 1]
                    nc.sync.dma_start(ull[:, a:b], ull_d[:, s0 + a : s0 + b, :, :])
                # all W-path sigmoids first: the in-order ACT queue must
                # not park a ptp sigmoid (waiting on the later ull DMA) in
                # front of the opE/opO halves that gate the W-build
                for h in range(nsplit):
                    a, b = bounds[h], bounds[h + 1]
                    ha, hb = a // 2, b // 2
                    # o_s(t) = sigmoid(signed logit); host pre-applied the signs
                    nc.scalar.activation(
                        opE[:, :, ha:hb],
                        zc[:, ha:hb, 0].rearrange("p ck s g -> p s ck g"),
                        AF.Sigmoid,
                    )
                    nc.scalar.activation(
                        opO[:, :, ha:hb],
                        zc[:, ha:hb, 1].rearrange("p ck s g -> p s ck g"),
                        AF.Sigmoid,
                    )
                for h in range(nsplit):
                    a, b = bounds[h], bounds[h + 1]
                    # true-outcome probs for predictions: [P(c|0), P(c|1)],
                    # written k-major so the Pool preds ops see 2-D-free APs
                    ca_, cb_ = a // K, b // K
                    for s in range(2):
                        nc.scalar.activation(
                            ptp[:, s, :, ca_:cb_, :],
                            ull[:, a:b, s, :]
                            .rearrange("p (c k) g -> p k c g", k=K),
                            AF.Sigmoid,
                        )
                obs[seg] = (ptp, We, Wo, opE, opO)

            def phase_ln(seg):
                """Ln + store for segment seg — emitted two segments late so
                the strictly in-order ACT engine never blocks the next
                segments' sigmoids behind a Ln that waits on DVE."""
                s0 = seg * SEG
                num, den = fin.pop(seg)
                # q = num * recip(den), deferred here (2 segments late) so the
                # recip never parks in the DVE wait queue blocking its SEQ
                dflat = den[:].rearrange("p k c g -> p (k c g)")
                nc.vector.reciprocal_approx_fast(dflat, dflat)
                pool_tt(den[:], num[:], den[:], OP.mult)
                # f16 output tile; the gpsimd (SWDGE) store casts to f32 in
                # the DMA — halves SBUF and store traffic, rel err +2.4e-4
                out_t = io.tile([P, SEG, 2, G], F16, tag="out")
                nsp = 2 if seg == NSEG - 1 else 1
                bounds = [SEG * h // nsp for h in range(nsp + 1)]
                for hh in range(nsp):
                    a, b = bounds[hh], bounds[hh + 1]
                    h0, h1 = a // K, b // K
                    qv = den[:, :, h0:h1, :].rearrange("p k c g -> p c k g")
                    nc.scalar.activation(out_t[:, a:b, 1, :], qv, AF.Ln)
                    nc.scalar.activation(out_t[:, a:b, 0, :], qv, AF.Ln,
                                         scale=-1.0, bias=1.0)
                    nc.gpsimd.dma_start(out_d[:, s0 + a : s0 + b, :, :], out_t[:, a:b])

            ctx = {}       # per-seg fold products for the tail phase

            def phase_fold(seg):
                """Chunk-product tree for segment seg as a list of closures.
                A_c = W_0..W_9 via pairwise tree, all fp16 2x on DVE."""
                ptp, We, Wo, opE, opO = obs[seg]
                TMp = work.tile([P, 2, 2, 2, CK, G], F16, tag="TMp")
                Pp = work.tile([P, 2, 2, CK, G], F16, tag="Pp")   # [a][s'][(ci)][g]
                TMs = work.tile([P, 2, 2, 2, CS, G], F16, tag="TMs")
                TMs1 = work.tile([P, 2, 2, 2, CS, G], F16, tag="TMs1")
                Q0 = qpool.tile([P, 2, 2, CS, G], F16, tag="Q0")
                Q1 = qpool.tile([P, 2, 2, CS, G], F16, tag="Q1")
                Rp = work.tile([P, 2, 2, CS, G], F16, tag="Rp")
                A = qpool.tile([P, 2, 2, CS, G], F16, tag="A")    # [a][s'][c][g]
                Pc = Pp[:].rearrange("p a u (c i) g -> p a u c i g", i=KH)
                ops = []
                # the W-builds are fully ready at block start (sigmoids ran an
                # era earlier) -- ideal pads for the interleaved serial chain
                for W_t, o_t in ((We, opE), (Wo, opO)):
                    ops.append(lambda W_t=W_t, o_t=o_t: nc.vector.tensor_tensor(
                        W_t[:],
                        o_t[:].unsqueeze(2).broadcast_to((P, 2, 2, CK, G)),
                        Tp2[:].unsqueeze(3).broadcast_to((P, 2, 2, CK, G)),
                        OP.mult,
                    ))
                ops.append(lambda: nc.vector.tensor_tensor(
                    TMp[:],
                    We[:].unsqueeze(3).broadcast_to((P, 2, 2, 2, CK, G)),
                    Wo[:].unsqueeze(1).broadcast_to((P, 2, 2, 2, CK, G)),
                    OP.mult,
                ))
                ops.append(lambda: nc.vector.tensor_tensor(
                    Pp[:], TMp[:, :, 0], TMp[:, :, 1], OP.add))
                for Qj, TMj, i0 in ((Q0, TMs, 0), (Q1, TMs1, 2)):
                    for a in range(2):
                        for m in range(2):
                            ops.append(lambda TMj=TMj, a=a, m=m, i0=i0:
                                nc.vector.tensor_tensor(
                                    TMj[:, a, m],
                                    Pc[:, a, m, :, i0].unsqueeze(1)
                                    .broadcast_to((P, 2, CS, G)),
                                    Pc[:, m, :, :, i0 + 1], OP.mult,
                                ))
                    ops.append(lambda Qj=Qj, TMj=TMj: nc.vector.tensor_tensor(
                        Qj[:], TMj[:, :, 0], TMj[:, :, 1], OP.add))
                for a in range(2):
                    ops.append(lambda a=a: nc.vector.tensor_tensor(
                        TMs[:, a],
                        Q0[:, a].unsqueeze(2).broadcast_to((P, 2, 2, CS, G)),
                        Q1[:], OP.mult,
                    ))
                ops.append(lambda: nc.vector.tensor_tensor(
                    Rp[:], TMs[:, :, 0], TMs[:, :, 1], OP.add))
                for a in range(2):
                    ops.append(lambda a=a: nc.vector.tensor_tensor(
                        TMs1[:, a],
                        Rp[:, a].unsqueeze(2).broadcast_to((P, 2, 2, CS, G)),
                        Pc[:, :, :, :, 4], OP.mult,
                    ))
                ops.append(lambda: nc.vector.tensor_tensor(
                    A[:], TMs1[:, :, 0], TMs1[:, :, 1], OP.add))
                ctx[seg] = (Q0, Q1, A)
                return ops

            def phase_tail(seg):
                """Serial recursion + recovery + predictions for segment seg
                as an ordered closure list (chain links get interleaved with
                the next segment's independent fold ops at emission time so
                the DVE wait queue never clogs its sequencer)."""
                c0 = seg * CS
                ptp, We, Wo, _opE, _opO = obs.pop(seg)
                Q0, Q1, A = ctx.pop(seg)
                ops = []

                # serial over chunk PAIRS: AA_c = A_{2c} @ A_{2c+1} (fp32 --
                # the double-chunk product can overflow fp16 in the tails)
                AAt = work.tile([P, 2, 2, 2, CS // 2, G], F32, tag="AAt")
                AA = work.tile([P, 2, 2, CS // 2, G], F32, tag="AA")
                for a in range(2):
                    for m in range(2):
                        ops.append(lambda a=a, m=m: nc.vector.tensor_tensor(
                            AAt[:, a, m],
                            A[:, a, m, 0::2].unsqueeze(1)
                            .broadcast_to((P, 2, CS // 2, G)),
                            A[:, m, :, 1::2], OP.mult,
                        ))
                ops.append(lambda: nc.vector.tensor_tensor(
                    AA[:], AAt[:, :, 0], AAt[:, :, 1], OP.add))
                sv = work.tile([P, CS // 2, 2, 2, G], F32, tag="sv")
                for cp in range(CS // 2):
                    cg = c0 + 2 * cp
                    ops.append(lambda cp=cp, cg=cg: nc.vector.tensor_tensor(
                        sv[:, cp],
                        starts[:, cg].unsqueeze(2).broadcast_to((P, 2, 2, G)),
                        AA[:, :, :, cp],
                        OP.mult,
                    ))
                    ops.append(lambda cp=cp, cg=cg: nc.vector.tensor_tensor(
                        starts[:, cg + 2], sv[:, cp, 0], sv[:, cp, 1], OP.add))
                # odd chunk starts: batched jumps off the even spine
                svo = work.tile([P, CS // 2, 2, 2, G], F32, tag="svo")
                for m in range(2):
                    ops.append(lambda m=m: nc.vector.tensor_tensor(
                        svo[:, :, m],
                        starts[:, c0 : c0 + CS : 2, m, :].unsqueeze(2)
                        .broadcast_to((P, CS // 2, 2, G)),
                        A[:, m, :, 0::2].rearrange("p u c g -> p c u g"),
                        OP.mult,
                    ))
                ops.append(lambda: nc.vector.tensor_tensor(
                    starts[:, c0 + 1 : c0 + CS + 1 : 2],
                    svo[:, :, 0], svo[:, :, 1], OP.add,
                ))
                # batched renormalize (chunks c0+1 .. c0+CS incl. carry)
                msum = work.tile([P, CS, G], F32, tag="msum")
                ops.append(lambda: nc.vector.tensor_tensor(
                    msum[:], starts[:, c0 + 1 : c0 + CS + 1, 0, :],
                    starts[:, c0 + 1 : c0 + CS + 1, 1, :], OP.add,
                ))
                ops.append(lambda: nc.vector.reciprocal_approx_fast(msum[:], msum[:]))
                ops.append(lambda: nc.vector.tensor_tensor(
                    starts[:, c0 + CS],
                    starts[:, c0 + CS],
                    msum[:, CS - 1].unsqueeze(1).broadcast_to((P, 2, G)),
                    OP.mult,
                ))

                # within-chunk recovery (fp16 2x, 3 chains round-robin)
                rec = work.tile([P, 2, K, CS, G], F16, tag="rec")   # [s][k][c][g]
                ops.append(lambda: nc.vector.tensor_scalar_add(
                    rec[:, :, 0, 0, :], starts[:, c0], 0.0))
                ops.append(lambda: nc.vector.tensor_tensor(
                    rec[:, :, 0, 1:, :],
                    starts[:, c0 + 1 : c0 + CS].rearrange("p c s g -> p s c g"),
                    msum[:, : CS - 1].unsqueeze(1).broadcast_to((P, 2, CS - 1, G)),
                    OP.mult,
                ))

                def wstep(j):
                    W_t = (We if j % 2 == 0 else Wo)[:]
                    return W_t.rearrange("p s u (c i) g -> p s u c i g", i=KH)[
                        :, :, :, :, j // 2
                    ]

                RRa = work.tile([P, 2, 2, CS, G], F16, tag="RRa")
                RRb = work.tile([P, 2, 2, CS, G], F16, tag="RRb")
                RRc = work.tile([P, 2, 2, CS, G], F16, tag="RRc")

                def rmul(scratch, j, src, mat):
                    def f():
                        nc.vector.tensor_tensor(
                            scratch[:],
                            rec[:, :, src].unsqueeze(2)
                            .broadcast_to((P, 2, 2, CS, G)),
                            wstep(j - 1) if mat is None else mat,
                            OP.mult,
                        )
                    return f

                def radd(scratch, j):
                    return lambda: nc.vector.tensor_tensor(
                        rec[:, :, j], scratch[:, 0], scratch[:, 1], OP.add)

                # chains: a = j1..3, b = join4(Q0), j5..7, c = join8(Q1), j9;
                # round-robin order keeps >=1 independent op between links
                plan = [
                    (RRb, 4, 0, Q0[:]), (RRa, 1, 0, None), (RRc, 8, 4, Q1[:]),
                    (RRa, 2, 1, None), (RRb, 5, 4, None), (RRa, 3, 2, None),
                    (RRb, 6, 5, None), (RRc, 9, 8, None), (RRb, 7, 6, None),
                ]
                for scratch, j, src, mat in plan:
                    ops.append(rmul(scratch, j, src, mat))
                    ops.append(radd(scratch, j))

                # predictions (k-major tiles; bulky elementwise on Pool)
                qp = work.tile([P, 2, K, CS, G], F16, tag="qp")     # [s][k][c][g]
                num = actb.tile([P, K, CS, G], F16, tag="num")
                den = actb.tile([P, K, CS, G], F32, tag="den")
                # den only needs rec -- emit it first so the deferred
                # reciprocal unblocks before the qp/num sequence finishes
                ops.append(lambda: pool_tt(den[:], rec[:, 0], rec[:, 1], OP.add))
                for s in range(2):
                    ops.append(lambda s=s: pool_tt(
                        qp[:, s], rec[:, s], ptp[:, s], OP.mult))
                ops.append(lambda: pool_tt(num[:], qp[:, 0], qp[:, 1], OP.add))
                fin[seg] = (num, den)
                return ops

            def emit_interleaved(fillers, chain):
                """1 filler per 2 chain ops (wait-queue depth is 4)."""
                fi, ci = 0, 0
                while fi < len(fillers) or ci < len(chain):
                    if fi < len(fillers):
                        fillers[fi](); fi += 1
                    for _ in range(2):
                        if ci < len(chain):
                            chain[ci](); ci += 1

            for seg in range(NSEG):
                phase_a(seg, nsplit=(5 if seg == 0 else 1))
                if seg >= 1:
                    emit_interleaved(
                        phase_fold(seg - 1),
                        phase_tail(seg - 2) if seg >= 2 else [],
                    )
                if seg >= 3:
                    phase_ln(seg - 3)
            emit_interleaved(phase_fold(NSEG - 1), phase_tail(NSEG - 2))
            for c in phase_tail(NSEG - 1):
                c()
            for s in range(max(0, NSEG - 3), NSEG):
                phase_ln(s)

    return nc


# ------------------------------------------------------------------
# Host-side full-problem wrapper
# ------------------------------------------------------------------

_B, _T, _K, _SEG = 16384, 500, 10, 100
_G = _B // (P * N_CORES)   # 16 groups per core

_cached = {}


def _build():
    if "nc" not in _cached:
        nc = bacc.Bacc(None, target_bir_lowering=False)
        emit_bkt(nc, G=_G, T=_T, K=_K, SEG=_SEG)
        nc.compile()
        _cached["nc"] = nc
    return _cached["nc"]


def _shard(arr, core):
    """(B,...) -> this core's (P, ..., G) permuted view, seq = g*128 + p."""
    rows = arr[core * P * _G : (core + 1) * P * _G]
    r = rows.reshape(_G, P, *arr.shape[1:])
    order = (1,) + tuple(range(2, r.ndim)) + (0,)
    return np.ascontiguousarray(r.transpose(order))


def kernel(corr, kc, problem, dynamics_logits_table, obs_logits_kc,
           obs_logits_problem, fastbkt_n):
    from concourse.bass_utils import run_bass_kernel_spmd

    corr = np.asarray(corr, dtype=np.float32)
    kc = np.asarray(kc).astype(np.int64)
    problem = np.asarray(problem).astype(np.int64)
    dyn_table = np.asarray(dynamics_logits_table, dtype=np.float32)
    obs_kc = np.asarray(obs_logits_kc, dtype=np.float32)
    obs_prob = np.asarray(obs_logits_problem, dtype=np.float32)

    B, T = corr.shape
    assert B == _B and T == _T, (B, T)

    # host gathers (traffic-neutral input marshaling)
    lls = obs_kc[kc][:, None, :] + obs_prob[problem]       # (B, T, 2) [lg, ls]
    dyn = dyn_table[kc]                                    # (B, 3)
    sgn = (corr * 2.0 - 1.0).astype(np.float32)            # (B, T)
    zll = np.empty((B, T, 2), np.float16)
    zll[:, :, 0] = sgn * lls[:, :, 0]
    zll[:, :, 1] = -sgn * lls[:, :, 1]
    ull = np.empty((B, T, 2), np.float16)
    ull[:, :, 0] = lls[:, :, 0]
    ull[:, :, 1] = -lls[:, :, 1]

    nc = _build()
    in_maps = []
    for core in range(N_CORES):
        in_maps.append({
            "zll": _shard(zll, core),
            "ull": _shard(ull, core),
            "dyn": _shard(dyn, core),
        })

    res = run_bass_kernel_spmd(
        nc, in_maps, core_ids=list(range(N_CORES)), **_cached.get("run_kwargs", {})
    )
    _cached["last_results"] = res

    out = np.empty((B, T, 2), np.float32)
    for core in range(N_CORES):
        o = res.results[core]["out"]                       # (P, T, 2, G)
        rows = o.transpose(3, 0, 1, 2).reshape(P * _G, T, 2)
        out[core * P * _G : (core + 1) * P * _G] = rows
    return out
